# revision 13
# baseline (speedup 1.0000x reference)
"""Trainium2 Bass kernel for nn_EnhancedQuantumLayer (6-qubit circuit, B=32768).

Reduction: AngleEmbedding (per-sample RX product state) + batch-independent
64x64 unitary U (weights only) + per-qubit PauliZ expectations:

    m_b   = kron_q [cos(a_q/2), sin(a_q/2)]           (real 64-vec, a = x*scale)
    A_b   = Cstat^T m_b                                (128-vec, re/im packed)
    EV_bq = sum_p sgn[p,q] * A_b[p]^2

Instruction-count-minimized pipeline (the platform charges a ~fixed cost per
engine instruction): per rep and core (4096 samples):
  sync : in-DMA (128,704), out-DMA (6,4096)
  ACT  : 1 fat Sin, 1 fat Square (PSUM->SBUF), 1 fat EV copy
  DVE  : k12 (fused pair-kron), k3, m12, mswz, StreamTranspose
  PE   : 8 projection matmuls + 8 sign matmuls (PSUM bank limit: 512 cols)
Constants (projection matrix, signs) are DMA'd once outside the rep loop.
"""
import math
from contextlib import ExitStack

import numpy as np

import concourse.bass as bass
import concourse.mybir as mybir
from concourse.bass_utils import run_bass_kernel_spmd

F32 = mybir.dt.float32
BF16 = mybir.dt.bfloat16
NQ = 6
NL = 6
B = 32768
NCORES = 8
BC = B // NCORES


# ---------------------------------------------------------------- host precompute
def _unitary64(weights):
    """Cc (64,64) complex: folded RX-embedding phases + circuit unitary."""
    w = np.asarray(weights, dtype=np.float64)
    phi, theta, omega = w[..., 0], w[..., 1], w[..., 2]
    ct, st = np.cos(0.5 * theta), np.sin(0.5 * theta)
    em = np.exp(-0.5j * (phi + omega))
    ep = np.exp(0.5j * (phi + omega))
    epm = np.exp(0.5j * (phi - omega))
    emp = np.exp(-0.5j * (phi - omega))

    state = np.eye(64, dtype=np.complex128).reshape((64,) + (2,) * NQ)

    def apply_1q(state, U, q):
        ax = q + 1
        s = np.moveaxis(state, ax, -1)
        s = np.einsum('ij,...j->...i', U, s)
        return np.moveaxis(s, -1, ax)

    def cnot(state, c, t):
        ca, ta = c + 1, t + 1
        s0 = np.take(state, 0, axis=ca)
        s1 = np.take(state, 1, axis=ca)
        t_in = ta - 1 if ta > ca else ta
        s1 = np.flip(s1, axis=t_in)
        return np.stack([s0, s1], axis=ca)

    for l in range(NL):
        for q in range(NQ):
            U = np.array([
                [em[l, q] * ct[l, q], -epm[l, q] * st[l, q]],
                [emp[l, q] * st[l, q], ep[l, q] * ct[l, q]],
            ])
            state = apply_1q(state, U, q)
        r = (l % (NQ - 1)) + 1
        for q in range(NQ):
            state = cnot(state, q, (q + r) % NQ)

    stateF = state.reshape(64, 64)
    e = np.arange(64)
    pc = np.array([bin(v).count('1') for v in e])
    phase = (-1j) ** pc
    return phase[:, None] * stateF            # (64_in_ref, 64_out)


def _host_const(weights):
    """cst (128, 134): [Cstat duplicated on both 64-halves | sgn]."""
    Cc = _unitary64(weights)
    # device contraction row j: bit->qubit map {5:q5,4:q4,3:q2,2:q3,1:q0,0:q1}
    j = np.arange(64)
    eref = (((j >> 5) & 1) * 1 + ((j >> 4) & 1) * 2 + ((j >> 3) & 1) * 8
            + ((j >> 2) & 1) * 4 + ((j >> 1) & 1) * 32 + (j & 1) * 16)
    Cdev = Cc[eref, :]                        # (64 j, 64 o)
    Cstat = np.empty((64, 128), np.float64)
    Cstat[:, 0::2] = Cdev.real                # projection p = 2o + 0
    Cstat[:, 1::2] = Cdev.imag                # projection p = 2o + 1
    p = np.arange(128)
    o = p >> 1
    sgn = np.stack([1.0 - 2.0 * ((o >> (5 - q)) & 1) for q in range(NQ)],
                   axis=1)                    # (128, 6)
    cst = np.zeros((128, 134), np.float32)
    cst[0:64, 0:128] = Cstat
    cst[64:128, 0:128] = Cstat
    cst[:, 128:134] = sgn
    return cst.astype(mybir.dt.np(BF16))


def _lane_sample_index():
    """SL[L, sb]: local sample index held by lane L at angle-block sb."""
    L = np.arange(128)
    h, jh, pl = L >> 6, (L >> 5) & 1, L & 31
    sb = np.arange(64)
    s2, tp, p_hi = sb >> 4, (sb >> 2) & 3, sb & 3
    return (1024 * p_hi[None, :] + 32 * pl[:, None]
            + 8 * s2[None, :] + 2 * tp[None, :] + h[:, None])


def _out_perm():
    """perm[c] = local sample index stored at device out column c."""
    c = np.arange(BC)
    h = c >> 11
    s2 = (c >> 9) & 3
    tp = (c >> 7) & 3
    p_hi = (c >> 5) & 3
    pl = c & 31
    return 1024 * p_hi + 32 * pl + 8 * s2 + 2 * tp + h


_SL = _lane_sample_index()
_PERM = _out_perm()


# ---------------------------------------------------------------- device program
def _build_bass(reps=1, unroll=4):
    """Per-engine hardware loops (Fori), `unroll` reps per loop body plus a
    static tail: the per-rep pipeline is 26 instructions; iterations
    synchronize with rep-indexed semaphore thresholds (standalone wait_ge
    supports register values)."""
    nc = bass.Bass()
    xin = nc.dram_tensor("xin", [128, 704], BF16, kind="ExternalInput")
    cin = nc.dram_tensor("cin", [128, 134], BF16, kind="ExternalInput")
    out = nc.dram_tensor("out", [NQ, BC], F32, kind="ExternalOutput")

    ctx = ExitStack()
    with ctx:
        sb = lambda nm, shape, dt=F32: ctx.enter_context(nc.sbuf_tensor(nm, shape, dt))
        sem = lambda nm: ctx.enter_context(nc.semaphore(name=nm))

        xt = sb("xt", [128, 704], BF16)
        scs = sb("scs", [128, 704], BF16)
        k12b = sb("k12b", [128, 512], BF16)
        k3b = sb("k3b", [128, 128], BF16)
        m12b = sb("m12b", [128, 1024], BF16)
        mswz = sb("mswz", [128, 2048], BF16)
        mtall0 = sb("mtall0", [128, 2048], BF16)
        mtall1 = sb("mtall1", [128, 2048], BF16)
        ppb = sb("ppb", [128, 4096], BF16)
        cstb = sb("cstb", [128, 134], BF16)
        evo = sb("evo", [NQ, 4096])
        PS = ctx.enter_context(nc.psum_tensor("PS", [128, 4096], F32))

        Sd, Sa, Sv, Sp, So = (sem("Sd"), sem("Sa"), sem("Sv"), sem("Sp"),
                              sem("So"))

        U = unroll if reps >= unroll else 1
        if U % 2:
            U = 1
        main = reps // U if U > 1 else 0
        tail = reps - main * U

        def emit(engine, body):
            """Emit `body(r, par)` for reps 1..reps; par = r%2 (static)."""
            if main:
                with engine.Fori(1, main + 1) as i:
                    for j in range(U):
                        body(U * i + (j + 1 - U), (j + 1 - U) % 2)
            for t in range(tail):
                r = main * U + t + 1
                body(r, r % 2)

        block = ctx.enter_context(nc.Block())

        @block.sync
        def _(sync):
            c0 = sync.dma_start(out=cstb.ap()[:, :], in_=cin[:, :])
            c0.then_inc(Sd, 16)

            def body(r, par):
                sync.wait_ge(Sv, 5 * r - 5)   # k3 of prev rep read xt/scs
                d = sync.dma_start(out=xt.ap()[:, :], in_=xin[:, :])
                d.then_inc(Sd, 16)
                sync.wait_ge(Sa, 3 * r)       # evcopy of this rep done
                o = sync.dma_start(out=out[:, :], in_=evo.ap()[:, :])
                o.then_inc(So, 16)

            emit(sync, body)
            sync.wait_ge(So, 16 * reps)

        @block.scalar
        def _(scalar):
            sfn = mybir.ActivationFunctionType.Sin
            sqf = mybir.ActivationFunctionType.Square

            def sin_r(r):
                scalar.wait_ge(Sd, 16 * r + 16)     # cin + in-DMA r done
                s_ = nc.scalar.activation(scs.ap()[:, :], xt.ap()[:, :], sfn)
                s_.then_inc(Sa, 1)

            def body(r, par=None):
                # software-pipelined ACT: sq/evcopy of rep r, sin of rep r+1
                scalar.wait_ge(Sp, 16 * r + 32)     # A-matmuls of rep r done
                q_ = nc.scalar.activation(ppb.ap()[:, :], PS.ap()[:, :], sqf)
                q_.then_inc(Sa, 1)
                scalar.wait_ge(Sp, 16 * r + 40)     # EV matmuls of rep r done
                scalar.wait_ge(So, 16 * r - 16)     # out-DMA of prev rep done
                e_ = nc.scalar.copy(evo.ap()[:, :], PS.ap()[0:NQ, :])
                e_.then_inc(Sa, 1)
                sin_r(r + 1)

            # prologue: sin of rep 1; loop/tail over reps 1..reps-1 with
            # lookahead sin; epilogue: sq/evcopy of the last rep.
            sin_r(1)
            n2 = reps - 1
            main2 = n2 // U if n2 >= U else 0
            U2 = U if main2 else 1
            main2 = n2 // U2
            tail2 = n2 - main2 * U2
            if main2:
                with scalar.Fori(1, main2 + 1) as i:
                    for j in range(U2):
                        body(U2 * i + (j + 1 - U2))
            for t in range(tail2):
                body(main2 * U2 + t + 1)
            scalar.wait_ge(Sp, 16 * reps + 32)
            q_ = nc.scalar.activation(ppb.ap()[:, :], PS.ap()[:, :], sqf)
            q_.then_inc(Sa, 1)
            scalar.wait_ge(Sp, 16 * reps + 40)
            scalar.wait_ge(So, 16 * reps - 16)
            e_ = nc.scalar.copy(evo.ap()[:, :], PS.ap()[0:NQ, :])
            e_.then_inc(Sa, 1)

        @block.vector
        def _(vector):
            def body(i, par):
                mt = [mtall0, mtall1][par]
                # k12: fused pair-kron for qubit pairs (0,1) and (2,3)
                vector.wait_ge(Sa, 3 * i - 2)       # sin of rep i done
                v = scs.ap()[:, 0:512].rearrange(
                    "p (hf sbj r) -> p sbj hf r", hf=2, r=2)
                i0 = v[:, :, :, 0:1].broadcast_to((128, 128, 2, 2))
                i1 = (v[:, :, :, 1:2]
                      .rearrange("p sbj hf one -> p sbj one hf")
                      .broadcast_to((128, 128, 2, 2)))
                ok = k12b.ap().rearrange(
                    "p (sbj hf0 hf1) -> p sbj hf0 hf1", hf0=2, hf1=2)
                t = nc.vector.tensor_mul(ok, i0, i1)
                t.then_inc(Sv, 1)
                # k3 = t4 (x) w   (qubit-5 factor via lane-parity bias)
                i0 = scs.ap()[:, 512:640].rearrange("p (hf sb) -> p sb hf",
                                                    hf=2)
                i1 = (scs.ap()[:, 640:704]
                      .rearrange("p (sb one) -> p sb one", one=1)
                      .broadcast_to((128, 64, 2)))
                o3 = k3b.ap().rearrange("p (sb b4) -> p sb b4", b4=2)
                t = nc.vector.tensor_mul(o3, i0, i1)
                t.then_inc(Sv, 1)
                # m12 = k1 (x) k2
                kv = k12b.ap().rearrange("p (sb j w) -> p sb j w", j=2, w=4)
                i0 = kv[:, :, 0:1, :].broadcast_to((128, 64, 4, 4))
                i1 = (kv[:, :, 1:2, :]
                      .rearrange("p sb one w -> p sb w one")
                      .broadcast_to((128, 64, 4, 4)))
                om = m12b.ap().rearrange("p (sb b32 b10) -> p sb b32 b10",
                                         b32=4, b10=4)
                t = nc.vector.tensor_mul(om, i0, i1)
                t.then_inc(Sv, 1)
                # mswz = m12 (x) k3  (block-swizzled for StreamTranspose)
                i0 = (m12b.ap().rearrange("p (sb w) -> p sb w", w=16)
                      .unsqueeze(2).broadcast_to((128, 64, 2, 16)))
                i1 = (k3b.ap().rearrange("p (sb b4) -> p sb b4", b4=2)
                      .unsqueeze(3).broadcast_to((128, 64, 2, 16)))
                oM = mswz.ap().rearrange("p (sb b4 w) -> p sb b4 w",
                                         b4=2, w=16)
                t = nc.vector.tensor_mul(oM, i0, i1)
                t.then_inc(Sv, 1)
                # transpose: basis onto partitions
                vector.wait_ge(Sp, 16 * i)          # A-mms of rep i-2 done
                st_ = nc.vector.transpose(mt.ap()[:, :], mswz.ap()[:, :])
                st_.then_inc(Sv, 1)

            emit(vector, body)

        @block.tensor
        def _(tensor):
            tensor.sem_inc(Sp, 40)                  # threshold seed

            def body(i, par):
                mt = [mtall0, mtall1][par]
                tensor.wait_ge(Sv, 5 * i)           # transpose of rep i done
                tensor.wait_ge(Sa, 3 * i - 3)       # evcopy of prev rep done
                for k in range(8):
                    h, s4 = divmod(k, 4)
                    mm = nc.tensor.matmul(
                        PS.ap()[:, k * 512:(k + 1) * 512],
                        cstb.ap()[64 * h:64 * h + 64, 0:128],
                        mt.ap()[64 * h:64 * h + 64,
                                s4 * 512:(s4 + 1) * 512],
                        start=True, stop=True, skip_group_check=True,
                    )
                    if k == 7:
                        mm.then_inc(Sp, 8)
                tensor.wait_ge(Sa, 3 * i - 1)       # square of rep i done
                for j in range(8):
                    mm = nc.tensor.matmul(
                        PS.ap()[0:NQ, j * 512:(j + 1) * 512],
                        cstb.ap()[:, 128:134],
                        ppb.ap()[:, j * 512:(j + 1) * 512],
                        start=True, stop=True, skip_group_check=True,
                    )
                    if j == 7:
                        mm.then_inc(Sp, 8)

            emit(tensor, body)

    return nc


_CACHE = {}


def _get_nc():
    if "nc" not in _CACHE:
        _CACHE["nc"] = _build_bass(reps=1)
    return _CACHE["nc"], _PERM


# ---------------------------------------------------------------- entry point
def _make_in_maps(x, weights, scale):
    x = np.asarray(x, dtype=np.float32)
    cst = _host_const(weights)
    hs = 0.5 * float(np.asarray(scale).reshape(-1)[0])
    a = x * hs                                 # half-angles
    L = np.arange(128)
    jh = (L >> 5) & 1
    wbias = np.where(jh == 0, math.pi / 2, 0.0).astype(np.float32)
    HP = np.float32(math.pi / 2)
    in_maps = []
    for k in range(NCORES):
        ak = a[k * BC:(k + 1) * BC]
        ang = ak[_SL]                          # (128, 64, 6)
        xs = np.empty((128, 704), np.float32)
        a4 = ang[:, :, 0:4].reshape(128, 256)  # col = sb*4 + qq
        xs[:, 0:256] = a4 + HP                 # cos half
        xs[:, 256:512] = a4                    # sin half
        xs[:, 512:576] = ang[:, :, 4] + HP
        xs[:, 576:640] = ang[:, :, 4]
        xs[:, 640:704] = ang[:, :, 5] + wbias[:, None]
        in_maps.append({"xin": xs.astype(mybir.dt.np(BF16)), "cin": cst})
    return in_maps


def kernel(x, weights, scale):
    nc, perm = _get_nc()
    in_maps = _make_in_maps(x, weights, scale)
    res = run_bass_kernel_spmd(nc, in_maps, list(range(NCORES))).results
    ev = np.empty((B, NQ), np.float32)
    for k in range(NCORES):
        ev[k * BC + perm, :] = res[k]["out"].T
    return ev


if __name__ == "__main__":
    rng = np.random.default_rng(0)
    x = rng.standard_normal((B, NQ)).astype(np.float32)
    weights = rng.uniform(0, 2 * np.pi, (NL, NQ, 3)).astype(np.float32)
    scale = np.array([0.1], np.float32)
    ev = kernel(x, weights, scale)
    print("out", ev.shape, ev.dtype, ev[:2])
